# revision 1
# baseline (speedup 1.0000x reference)
"""Trainium2 Bass kernel for nn_CrossAttention (B=4, C=256, H=W=64).

Sharding: 8 cores = (batch b, query-half h). Each core computes, for its
batch and its half of the query rows i (IH=2048):
  q = Wq x_i + bq        [32, 2048] stored 4x row-replicated as q4 [128, 2048]
  k = Wk x_f             [32, 4096] stored 4x row-replicated as k4 [128, 4096]
                         (bk dropped: constant-in-j shift is softmax-invariant)
  vT = (Wv x_f)^T        [4096, 256] bf16  (bv folded into bc_eff on host)
  S^T[j, i] = k_j . q_i  (transposed layout; row-replication lets two K=32
                          score matmuls run concurrently in PE row strips)
  E = exp(S^T) bf16      (no max subtraction: |S| <~ 30, exp safe in f32)
  att_raw[c, i] = sum_j vT[j, c] E[j, i]   (UN-normalized)
  r[i] = sum_j E[j, i]
  out[i] = (1/r) sum_c |r~ (Wcx x_i + bc) + Wca att_raw|   (r~ = bf16(r);
           the softmax 1/r normalization is algebraically deferred through
           the linear combine and pulled out of the abs-sum)

Engine budget per query block (~17.4us of PE work): exp on ACT ~16us, the
r partial sums as one DVE pair-add per group with accumulation on the DMA
engines (two alternating chains; DVE was the bottleneck when it held the
whole chain), partition fold of r as a single f32 ones-matmul into the 8th
PSUM bank, recip via reciprocal_approx_fast. The r tail for block ib is
EMITTED inside block ib+1's group loop so the in-order PE queue never
stalls on it (PE idle >3.4us re-throttles the clock to 1.2GHz via HAM).
PSUM: 4 score staging + 3 attended + 1 r = 8 banks.
"""

import numpy as np
import ml_dtypes

import concourse.bass as bass
import concourse.bacc as bacc
import concourse.tile as tile
import concourse.mybir as mybir
from concourse.bass_utils import run_bass_kernel_spmd

B, C, HH, WW = 4, 256, 64, 64
N = HH * WW          # 4096
CQK = 32
IH = N // 2          # 2048 query rows per core
NCORES = 8
NJC = N // 128       # 32 key-dim 128-chunks
NG = NJC // 2        # 16 groups of 2 key-chunks

F32 = mybir.dt.float32
F32R = mybir.dt.float32r
BF16 = mybir.dt.bfloat16
AF = mybir.ActivationFunctionType
ALU = mybir.AluOpType


def build_program(nc, tc):
    # ---- DRAM I/O ------------------------------------------------------
    dram = {}
    for name, shape, dt in [
        ("x1f", [2, 128, N], BF16), ("x2f", [2, 128, N], BF16),
        ("x1i", [2, 128, IH], BF16), ("x2i", [2, 128, IH], BF16),
        ("wqt", [2, 128, 128], BF16), ("wkt", [2, 128, 128], BF16),
        ("wvt", [2, 128, C], BF16),
        ("wctx", [2, 128, C], BF16), ("wcta", [2, 128, C], BF16),
        ("bq", [128, 1], F32), ("bce", [1, 2, 128], BF16),
        ("onesc", [128, 1], F32R),
    ]:
        dram[name] = nc.dram_tensor(name, shape, dt, kind="ExternalInput").ap()
    out_d = nc.dram_tensor("out", [2, IH], F32, kind="ExternalOutput").ap()

    import contextlib
    with contextlib.ExitStack() as ctx:
        persist = ctx.enter_context(tc.tile_pool(name="persist", bufs=1))

        wq_sb = persist.tile([128, 2, 128], BF16, tag="wq")
        wk_sb = persist.tile([128, 2, 128], BF16, tag="wk")
        wv_sb = persist.tile([128, 2, C], BF16, tag="wv")
        wcx_sb = persist.tile([128, 2, C], BF16, tag="wcx")
        wca_sb = persist.tile([128, 2, C], BF16, tag="wca")
        bq_sb = persist.tile([128, 1], F32, tag="bq")
        bce_sb = persist.tile([1, 2, 128], BF16, tag="bce")
        ones_bf = persist.tile([128, 1], BF16, tag="ones")
        ones_f = persist.tile([128, 1], F32R, tag="onesf")
        x1i_sb = [persist.tile([128, IH], BF16, tag=f"x1i{kc}",
                               name=f"x1i{kc}") for kc in range(2)]

        # DMA issue order = need order: q-projection inputs first
        nc.sync.dma_start(out=bq_sb, in_=dram["bq"])
        for kc in range(2):
            nc.sync.dma_start(out=wq_sb[:, kc, :], in_=dram["wqt"][kc])
        for kc in range(2):
            nc.sync.dma_start(out=x1i_sb[kc], in_=dram["x1i"][kc])
        for w, t in [("wkt", wk_sb), ("wvt", wv_sb)]:
            for kc in range(2):
                nc.sync.dma_start(out=t[:, kc, :], in_=dram[w][kc])
        nc.vector.memset(ones_bf, 1.0)
        nc.sync.dma_start(out=ones_f, in_=dram["onesc"])

        # projection outputs; k4/vT split in j-halves for earlier consumption
        q4_sb = [persist.tile([128, IH], BF16, tag=f"q{i}", name=f"q{i}")
                 for i in range(2)]
        k4_sb = [[persist.tile([128, N // 2], BF16, tag=f"k{i}{h}",
                               name=f"k{i}{h}") for h in range(2)]
                 for i in range(2)]
        vT_sb = [[persist.tile([128, (NJC // 2) * C], BF16, tag=f"vt{i}{h}",
                               name=f"vt{i}{h}") for h in range(2)]
                 for i in range(2)]
        att_sb = [[persist.tile([128, IH], BF16, tag=f"att{br}{c2}",
                                name=f"att{br}{c2}") for c2 in range(2)]
                  for br in range(2)]
        # per (br, ib) softmax-denominator products, consumed by phase 2b
        rb_sb = [[persist.tile([128, 512], BF16, tag=f"rb{br}{ib}",
                               name=f"rb{br}{ib}") for ib in range(4)]
                 for br in range(2)]
        rlb_sb = [[persist.tile([1, 512], BF16, tag=f"rlb{br}{ib}",
                                name=f"rlb{br}{ib}") for ib in range(4)]
                  for br in range(2)]
        rr_sb = [[persist.tile([1, 512], F32, tag=f"rr{br}{ib}",
                               name=f"rr{br}{ib}") for ib in range(4)]
                 for br in range(2)]

        # ---- phase 1: projections -------------------------------------
        with tc.tile_pool(name="proj_sb", bufs=2) as proj_sb, \
             tc.tile_pool(name="ps_kq", bufs=3, space="PSUM") as ps_kq, \
             tc.tile_pool(name="ps_vt", bufs=2, space="PSUM") as ps_vt:

            # q4 projections (from islice inputs; bq folded via ACT bias)
            for xi in range(2):
                if xi == 0:
                    xi_sb = x1i_sb
                else:
                    xi_sb = [proj_sb.tile([128, IH], BF16, tag="x2i",
                                          name="x2i") for _ in range(2)]
                    for kc in range(2):
                        nc.sync.dma_start(out=xi_sb[kc], in_=dram["x2i"][kc])
                for ib in range(4):
                    sl = bass.ts(ib, 512)
                    qp = ps_kq.tile([128, 512], F32, tag="kq", name="qp")
                    for kc in range(2):
                        nc.tensor.matmul(qp, wq_sb[:, kc, :], xi_sb[kc][:, sl],
                                         start=(kc == 0), stop=(kc == 1))
                    nc.scalar.activation(q4_sb[xi][:, sl], qp, AF.Identity,
                                         bias=bq_sb)

            # k4 and vT projections, x2 first (branch 0 needs vT2)
            for xi, xf_name in [(1, "x2f"), (0, "x1f")]:
                for jh in range(2):
                    xf_t = proj_sb.tile([128, 2, IH], BF16, tag="xf",
                                        name="xf")
                    for kc in range(2):
                        nc.sync.dma_start(
                            out=xf_t[:, kc, :],
                            in_=dram[xf_name][kc][:, jh * IH:(jh + 1) * IH])
                    for jb in range(4):
                        sl = bass.ts(jb, 512)
                        kp = ps_kq.tile([128, 512], F32, tag="kq", name="kp")
                        for kc in range(2):
                            nc.tensor.matmul(kp, wk_sb[:, kc, :],
                                             xf_t[:, kc, sl],
                                             start=(kc == 0), stop=(kc == 1))
                        nc.scalar.activation(k4_sb[xi][jh][:, sl], kp, AF.Copy)
                    for g in range(4):
                        vtp = ps_vt.tile([128, 4, C], F32, tag="vt",
                                         name="vtp")
                        for s in range(4):
                            jsub = g * 4 + s
                            for kc in range(2):
                                nc.tensor.matmul(
                                    vtp[:, s, :],
                                    xf_t[:, kc, bass.ts(jsub, 128)],
                                    wv_sb[:, kc, :],
                                    start=(kc == 0), stop=(kc == 1))
                        nc.vector.tensor_copy(
                            vT_sb[xi][jh][:, bass.ds(g * 4 * C, 4 * C)],
                            vtp.rearrange("p a c -> p (a c)"))
            nc.sync.dma_start(out=bce_sb, in_=dram["bce"])
            for kc in range(2):
                nc.sync.dma_start(out=wcx_sb[:, kc, :], in_=dram["wctx"][kc])
                nc.sync.dma_start(out=wca_sb[:, kc, :], in_=dram["wcta"][kc])

        # r-fold PSUM bank and r-accumulation SBUF outlive 2a (the final
        # block's fold is emitted from inside phase 2b)
        ps_rp = ctx.enter_context(
            tc.tile_pool(name="ps_rp", bufs=1, space="PSUM"))
        racc_pool = ctx.enter_context(tc.tile_pool(name="racc_sb", bufs=1))

        # ---- phase 2a: attention (att_sb <- raw attended; r products) --
        # Software-pipelined emission: the PE queue is strictly in-order,
        # so scores/exp for unit n+1 are emitted BEFORE attended(n) -- the
        # PE runs the next scores while attended(n) waits on exp(n),
        # instead of idling ~1us every group (which also re-throttles HAM).
        with tc.tile_pool(name="attn_sb", bufs=1) as attn_sb, \
             tc.tile_pool(name="ps_att", bufs=1, space="PSUM") as ps_att, \
             tc.tile_pool(name="ps_st", bufs=1, space="PSUM") as ps_st:

            def r_tail(br, ib, racc_v, racc_d):
                """Fold r partials + derive 2b products. Emitted deferred
                (from inside a later block's group loop) so no PE
                instruction ever waits on the accumulate chains."""
                rp = ps_rp.tile([1, 512], F32, tag="rp", bufs=1, name="rp")
                nc.tensor.matmul(rp, ones_f, racc_v, start=True, stop=False)
                nc.tensor.matmul(rp, ones_f, racc_d, start=False, stop=True)
                # 1/r (f32, ~18-bit) straight from PSUM
                nc.vector.reciprocal_approx_fast(rr_sb[br][ib], rp)
                # r line -> bf16 (ACT), then partition-broadcast for the
                # 2b x1-prescale
                nc.scalar.activation(rlb_sb[br][ib], rp, AF.Copy)
                nc.gpsimd.partition_broadcast(rb_sb[br][ib], rlb_sb[br][ib])

            units = [(br, ib, g) for br in range(2) for ib in range(4)
                     for g in range(NG)]
            state = {}   # (br, ib) -> dict with attp/racc/est-per-g

            def emit_scores_exp(br, ib, g):
                q4, k4 = q4_sb[br], k4_sb[br]
                isl = bass.ts(ib, 512)
                st = state.setdefault((br, ib), {"est": {}})
                if g == 0:
                    st["attp"] = [ps_att.tile([128, 512], F32, tag="attp",
                                              bufs=3, name=f"attp{c2}")
                                  for c2 in range(2)]
                    st["racc_v"] = None
                    st["racc_d"] = racc_pool.tile([128, 512], F32R,
                                                  tag="raccd", bufs=2,
                                                  name="racc_d")
                jcs = (2 * g, 2 * g + 1)
                jh = g // (NG // 2)
                jloc = [jc - jh * (NJC // 2) for jc in jcs]
                stp = ps_st.tile([128, 2, 512], F32, tag="stp",
                                 bufs=2, name="stp")
                for t in range(2):
                    nc.tensor.matmul(
                        stp[:, t, :],
                        k4[jh][32 * t:32 * (t + 1), bass.ts(jloc[t], 128)],
                        q4[32 * t:32 * (t + 1), isl],
                        start=True, stop=True, tile_position=(32 * t, 0))
                est = attn_sb.tile([128, 2, 512], BF16, tag="est",
                                   bufs=8, name="est")
                nc.scalar.activation(est.rearrange("p a n -> p (a n)"),
                                     stp.rearrange("p a n -> p (a n)"),
                                     AF.Exp)
                st["est"][g] = est

            pending = None
            emit_scores_exp(0, 0, 0)
            for n, (br, ib, g) in enumerate(units):
                if n + 1 < len(units):
                    emit_scores_exp(*units[n + 1])
                st = state[(br, ib)]
                est, attp = st["est"].pop(g), st["attp"]
                jcs = (2 * g, 2 * g + 1)
                jh = g // (NG // 2)
                jloc = [jc - jh * (NJC // 2) for jc in jcs]
                vT = vT_sb[1 - br]
                for t in range(2):
                    for c2 in range(2):
                        nc.tensor.matmul(
                            attp[c2],
                            vT[jh][:, bass.ds(jloc[t] * C + c2 * 128, 128)],
                            est[:, t, :],
                            start=(g == 0 and t == 0),
                            stop=(g == NG - 1 and t == 1))
                if g == NG - 1:
                    # release attp first: casts jump the DVE queue ahead of
                    # this group's r ops so the next block's attended can
                    # allocate from the 3-deep attp ring without waiting
                    isl = bass.ts(ib, 512)
                    for c2 in range(2):
                        nc.vector.tensor_copy(att_sb[br][c2][:, isl],
                                              attp[c2])
                # r partials: pair-sum the two strips; even groups chain on
                # the DVE (ping-pong), odd groups on one DMA-accumulate
                # chain -- keeps both DVE and gpsimd under the PE group time
                if g % 2 == 0:
                    rtmp = racc_pool.tile([128, 512], BF16, tag="rtmpb",
                                          bufs=2, name="rtmp_b")
                    nc.vector.tensor_tensor(rtmp, est[:, 0, :],
                                            est[:, 1, :], ALU.add)
                    rv = racc_pool.tile([128, 512], F32R, tag="raccv",
                                        bufs=2, name="racc_v")
                    if st["racc_v"] is None:
                        nc.vector.tensor_copy(rv, rtmp)
                    else:
                        nc.vector.tensor_tensor(rv, st["racc_v"], rtmp,
                                                ALU.add)
                    st["racc_v"] = rv
                elif g == 1:
                    nc.vector.tensor_tensor(st["racc_d"], est[:, 0, :],
                                            est[:, 1, :], ALU.add)
                else:
                    rtmp = racc_pool.tile([128, 512], F32R, tag="rtmp",
                                          bufs=3, name="rtmp")
                    nc.vector.tensor_tensor(rtmp, est[:, 0, :],
                                            est[:, 1, :], ALU.add)
                    nc.gpsimd.dma_start(out=st["racc_d"], in_=rtmp,
                                        accum_op=ALU.add)
                if g == 8 and pending is not None:
                    r_tail(*pending)
                    pending = None
                if g == NG - 1:
                    pending = (br, ib, st["racc_v"], st["racc_d"])
                    del state[(br, ib)]
            last_pending = pending

        # ---- phase 2b: combines, back-to-back on the PE ----------------
        # Same one-ahead trick: outp/osb of block n are emitted after the
        # cp matmuls of block n+1 so the PE never waits on the Abs ACT.
        with tc.tile_pool(name="cmb_sb", bufs=1) as cmb_sb, \
             tc.tile_pool(name="ps_cmb", bufs=1, space="PSUM") as ps_cmb:

            def emit_cp(br, ib):
                isl = bass.ts(ib, 512)
                x1r = cmb_sb.tile([128, 2, 512], BF16, tag="x1r",
                                  bufs=2, name="x1r")
                for kc in range(2):
                    nc.vector.tensor_tensor(x1r[:, kc, :],
                                            x1i_sb[kc][:, isl],
                                            rb_sb[br][ib], ALU.mult)
                absb = []
                for c2 in range(2):
                    cp = ps_cmb.tile([128, 512], F32, tag="cp",
                                     bufs=3, name="cp")
                    for kc in range(2):
                        nc.tensor.matmul(cp, wcx_sb[:, kc, bass.ts(c2, 128)],
                                         x1r[:, kc, :],
                                         start=(kc == 0), stop=False)
                    nc.tensor.matmul(cp, bce_sb[:, c2, :], rlb_sb[br][ib],
                                     start=False, stop=False)
                    for kc in range(2):
                        nc.tensor.matmul(cp, wca_sb[:, kc, bass.ts(c2, 128)],
                                         att_sb[br][kc][:, isl],
                                         start=False, stop=(kc == 1))
                    ab = cmb_sb.tile([128, 512], BF16, tag="absb",
                                     bufs=4, name="absb")
                    nc.scalar.activation(ab, cp, AF.Abs)
                    absb.append(ab)
                return absb

            def emit_out(br, ib, absb):
                isl = bass.ts(ib, 512)
                outp = ps_cmb.tile([1, 512], F32, tag="outp", bufs=2,
                                   name="outp")
                for c2 in range(2):
                    nc.tensor.matmul(outp, ones_bf, absb[c2],
                                     start=(c2 == 0), stop=(c2 == 1))
                osb = cmb_sb.tile([1, 512], F32, tag="osb", bufs=2,
                                  name="osb")
                nc.vector.tensor_tensor(osb, outp, rr_sb[br][ib], ALU.mult)
                nc.sync.dma_start(out=out_d[br:br + 1, isl], in_=osb)

            blocks = [(br, ib) for br in range(2) for ib in range(4)]
            prev = None
            for bi, (br, ib) in enumerate(blocks):
                absb = emit_cp(br, ib)
                if prev is not None:
                    emit_out(*prev)
                prev = (br, ib, absb)
                if bi == 2:
                    r_tail(*last_pending)
            emit_out(*prev)


_NC_CACHE = {}


def _get_nc():
    if "nc" not in _NC_CACHE:
        nc = bacc.Bacc("TRN2", debug=False, enable_asserts=False,
                       target_bir_lowering=False, enable_partition_id=False)
        with tile.TileContext(nc) as tc:
            build_program(nc, tc)
        nc.compile()
        _NC_CACHE["nc"] = nc
    return _NC_CACHE["nc"]


def host_inputs(x1, x2, Wq, bq, Wk, bk, Wv, bv, Wc, bc):
    """Build the 8 per-core input maps (host-side sharding/layout only)."""
    f = np.float32
    bf = ml_dtypes.bfloat16
    x1 = np.asarray(x1, f); x2 = np.asarray(x2, f)
    Wq = np.asarray(Wq, f); bq = np.asarray(bq, f)
    Wk = np.asarray(Wk, f)
    Wv = np.asarray(Wv, f); bv = np.asarray(bv, f)
    Wc = np.asarray(Wc, f); bc = np.asarray(bc, f)

    # 4x row-replicated q/k projection weights -> q4/k4 [128, n] layouts
    Wq4 = np.tile(Wq, (4, 1))            # [128, 256]
    Wk4 = np.tile(Wk, (4, 1))
    wqt = np.ascontiguousarray(Wq4.T.reshape(2, 128, 128)).astype(bf)
    wkt = np.ascontiguousarray(Wk4.T.reshape(2, 128, 128)).astype(bf)
    bq4 = np.tile(bq, 4).reshape(128, 1).copy()
    wvt = np.ascontiguousarray(Wv.T.reshape(2, 128, C)).astype(bf)
    WcT = np.ascontiguousarray(Wc.T)     # [512, 256]
    wctx = WcT[:C].reshape(2, 128, C).astype(bf)
    wcta = WcT[C:].reshape(2, 128, C).astype(bf)
    bce = (bc + Wc[:, C:] @ bv).reshape(1, 2, 128).astype(bf)

    in_maps = []
    for core in range(NCORES):
        b, h = divmod(core, 2)
        x1f = x1[b].reshape(C, N).reshape(2, 128, N)
        x2f = x2[b].reshape(C, N).reshape(2, 128, N)
        in_maps.append({
            "x1f": np.ascontiguousarray(x1f).astype(bf),
            "x2f": np.ascontiguousarray(x2f).astype(bf),
            "x1i": np.ascontiguousarray(
                x1f[:, :, h * IH:(h + 1) * IH]).astype(bf),
            "x2i": np.ascontiguousarray(
                x2f[:, :, h * IH:(h + 1) * IH]).astype(bf),
            "wqt": wqt, "wkt": wkt, "wvt": wvt,
            "wctx": wctx, "wcta": wcta,
            "bq": bq4, "bce": bce,
            "onesc": np.ones((128, 1), f),
        })
    return in_maps


def assemble(results):
    """results: list of 8 dicts with 'out' [2, IH] -> (out1, out2) full."""
    outs = []
    for row in range(2):
        full = np.empty((B, 1, HH, WW), np.float32)
        for b in range(B):
            half0 = results[2 * b]["out"][row]
            half1 = results[2 * b + 1]["out"][row]
            full[b, 0] = np.concatenate([half0, half1]).reshape(HH, WW)
        outs.append(full)
    return outs[0], outs[1]


def kernel(x1, x2, Wq, bq, Wk, bk, Wv, bv, Wc, bc):
    in_maps = host_inputs(x1, x2, Wq, bq, Wk, bk, Wv, bv, Wc, bc)
    nc = _get_nc()
    res = run_bass_kernel_spmd(nc, in_maps, core_ids=list(range(NCORES)))
    return assemble(res.results)



# revision 11
# speedup vs baseline: 1.1715x; 1.1715x over previous
"""Trainium2 Bass kernel for nn_CrossAttention (B=4, C=256, H=W=64).

Sharding: 8 cores = (batch b, branch br). Each core computes its
branch's full 4096-query attention + combine for one batch.

Single fused pipeline:
  - 8 i-blocks of 512 rows x 16 windows of 2 key-chunks. Per window:
    2 strip score matmuls (K=32, tile_position rows 0/32) -> stp PSUM
    f32 [128,2,512]; one ACT Exp -> est bf16 [128,2,512] (SBUF ring).
  - r[i]: DVE pair-sums est planes (bf16 4x mode), then 8 exact ones-
    matmul folds per block into a persistent PSUM row.
  - Wca is folded into the v-projection on the host (W2 = Wca @ Wv),
    so "attended" lives in combine space: the 32 bf16 matmuls per
    (block, c2) accumulate DIRECTLY inside the combine's PSUM group,
    emitted one block later (est ring holds 2 blocks), then
    cp = sum_w,t W2vT^T est + bce*r + Wcx (x1*r); out = sum_c|cp|/r
    via |.| (DVE abs_max) and a ones-matmul into the outp PSUM row.
  - No attended PSUM banks, no separate phases: PSUM = stp ring 2x2
    + combine/projection ring 3 + r/outp row pair 1 = 8 banks exactly.
  - q/k/vT2 projections are "pieces" interleaved into the window loop
    on the combine banks (block 0 has no combine work -> pieces live
    there); xa (own branch) loads first so scores start immediately,
    xb (other branch, for vT2) is only needed once combines begin.
"""

import numpy as np
import ml_dtypes

import concourse.bass as bass
import concourse.bacc as bacc
import concourse.tile as tile
import concourse.mybir as mybir
from concourse.bass_utils import run_bass_kernel_spmd

B, C, HH, WW = 4, 256, 64, 64
N = HH * WW          # 4096
CQK = 32
NCORES = 8
NCH = N // 128       # 32 key chunks
NWIN = 16            # windows per block (2 chunks each)
NBLK = 8             # i-blocks of 512

F32 = mybir.dt.float32
BF16 = mybir.dt.bfloat16
AF = mybir.ActivationFunctionType
ALU = mybir.AluOpType


def build_program(nc, tc):
    dram = {}
    for name, shape, dt in [
        ("xa", [2, 128, N], BF16), ("xb", [2, 128, N], BF16),
        ("xc", [2, 128, N], BF16),
        ("wqt", [2, 128, 128], BF16), ("wkt", [2, 128, 128], BF16),
        ("wvt2", [2, 128, C], BF16), ("wctx", [2, 128, C], BF16),
        ("bq", [128, 1], F32), ("bce", [1, 2, 128], BF16),
    ]:
        dram[name] = nc.dram_tensor(name, shape, dt, kind="ExternalInput").ap()
    out_d = nc.dram_tensor("out", [1, N], F32, kind="ExternalOutput").ap()

    import contextlib
    with contextlib.ExitStack() as ctx:
        persist = ctx.enter_context(tc.tile_pool(name="persist", bufs=1))

        wq_sb = persist.tile([128, 2, 128], BF16, tag="wq")
        wk_sb = persist.tile([128, 2, 128], BF16, tag="wk")
        wv2_sb = persist.tile([128, 2, C], BF16, tag="wv2")
        wcx_sb = persist.tile([128, 2, C], BF16, tag="wcx")
        bq_sb = persist.tile([128, 1], F32, tag="bq")
        bce_sb = persist.tile([1, 2, 128], BF16, tag="bce")
        onesb_sb = persist.tile([128, 1], BF16, tag="onesb")
        warm_sb = persist.tile([128, 1], BF16, tag="warm")
        xa_sb = [persist.tile([128, N], BF16, tag=f"xa{kc}",
                              name=f"xa{kc}") for kc in range(2)]
        xb_sb = [persist.tile([128, N], BF16, tag=f"xb{kc}",
                              name=f"xb{kc}") for kc in range(2)]
        xc_sb = [persist.tile([128, N], BF16, tag=f"xc{kc}",
                              name=f"xc{kc}") for kc in range(2)]
        q4_sb = persist.tile([128, N], BF16, tag="q4")
        k4_sb = persist.tile([128, N], BF16, tag="k4")
        vT2_sb = persist.tile([128, NWIN, 2, C], BF16, tag="vt2")

        # ---- input DMAs, in consumption order -------------------------
        nc.sync.dma_start(out=bq_sb, in_=dram["bq"])
        for kc in range(2):
            nc.sync.dma_start(out=wq_sb[:, kc, :], in_=dram["wqt"][kc])
        for kc in range(2):
            nc.sync.dma_start(out=wk_sb[:, kc, :], in_=dram["wkt"][kc])
        nc.sync.dma_start(out=bce_sb, in_=dram["bce"])
        for jh in range(2):          # own-branch input: q/k projections
            for kc in range(2):
                nc.sync.dma_start(
                    out=xa_sb[kc][:, jh * (N // 2):(jh + 1) * (N // 2)],
                    in_=dram["xa"][kc][:, jh * (N // 2):(jh + 1) * (N // 2)])
        for kc in range(2):
            nc.sync.dma_start(out=wv2_sb[:, kc, :], in_=dram["wvt2"][kc])
        for kc in range(2):
            nc.sync.dma_start(out=wcx_sb[:, kc, :], in_=dram["wctx"][kc])
        for kc in range(2):          # other branch: vT2 projection
            nc.sync.dma_start(out=xb_sb[kc], in_=dram["xb"][kc])
        for kc in range(2):          # x1 for the combine
            nc.sync.dma_start(out=xc_sb[kc], in_=dram["xc"][kc])
        nc.vector.memset(onesb_sb, 1.0)
        nc.scalar.activation(warm_sb, onesb_sb, AF.Exp)  # pull ACT table load

        # ---- pools -----------------------------------------------------
        ps_stp = ctx.enter_context(
            tc.tile_pool(name="ps_stp", bufs=2, space="PSUM"))
        ps_cmb = ctx.enter_context(
            tc.tile_pool(name="ps_cmb", bufs=2, space="PSUM"))
        ps_rp = ctx.enter_context(
            tc.tile_pool(name="ps_rp", bufs=1, space="PSUM"))
        sb = ctx.enter_context(tc.tile_pool(name="work_sb", bufs=1))

        rp_t = ps_rp.tile([1, 512], F32, tag="rp")
        op_t = ps_rp.tile([1, 512], F32, tag="outp")

        # ---- projection pieces (run on the cmb banks) ------------------
        def q_piece(ib, act=False):
            qp = ps_cmb.tile([128, 512], F32, tag="cmb", bufs=2, name="qp")
            for kc in range(2):
                nc.tensor.matmul(qp, wq_sb[:, kc, :],
                                 xa_sb[kc][:, bass.ts(ib, 512)],
                                 start=(kc == 0), stop=(kc == 1))
            nc.scalar.activation(q4_sb[:, bass.ts(ib, 512)], qp,
                                 AF.Identity, bias=bq_sb)

        def k_piece(jb, act=False):
            kp = ps_cmb.tile([128, 512], F32, tag="cmb", bufs=2, name="kp")
            for kc in range(2):
                nc.tensor.matmul(kp, wk_sb[:, kc, :],
                                 xa_sb[kc][:, bass.ts(jb, 512)],
                                 start=(kc == 0), stop=(kc == 1))
            dst = k4_sb[:, bass.ts(jb, 512)]
            if act:
                nc.scalar.activation(dst, kp, AF.Copy)
            else:
                nc.vector.tensor_copy(dst, kp)

        def v_piece(p, act=False):
            vp = ps_cmb.tile([128, 512], F32, tag="cmb", bufs=2,
                             name="vp").rearrange("q (s c) -> q s c", s=2)
            for s in range(2):
                jc = 2 * p + s
                for kc in range(2):
                    nc.tensor.matmul(vp[:, s, :],
                                     xb_sb[kc][:, bass.ts(jc, 128)],
                                     wv2_sb[:, kc, :],
                                     start=(kc == 0), stop=(kc == 1))
            dst = vT2_sb[:, p, :, :]
            if act:
                nc.scalar.activation(dst.rearrange("q s c -> q (s c)"),
                                     vp.rearrange("q s c -> q (s c)"),
                                     AF.Copy)
            else:
                nc.vector.tensor_copy(dst, vp)

        # ---- combine pieces for a finished block -----------------------
        bstate = {}

        def cp_piece(n, c2):
            st = bstate[n]
            u = ps_cmb.tile([128, 512], F32, tag="cmb", bufs=2, name="cp")
            csl = bass.ds(c2 * 128, 128)
            first = True
            for w in range(NWIN):
                for t in range(2):
                    nc.tensor.matmul(u, vT2_sb[:, w, t, csl],
                                     st["est"][w][:, t, :],
                                     start=first, stop=False)
                    first = False
            nc.tensor.matmul(u, bce_sb[:, c2, :], st["rlb"],
                             start=False, stop=False)
            for kc in range(2):
                nc.tensor.matmul(u, wcx_sb[:, kc, csl], st["x1r"][:, kc, :],
                                 start=False, stop=(kc == 1))
            ab = sb.tile([128, 512], BF16, tag="absb", bufs=4, name="absb")
            nc.scalar.activation(ab, u, AF.Abs)
            st.setdefault("absb", []).append(ab)

        def outp_piece(n):
            st = bstate[n]
            for c2 in range(2):
                nc.tensor.matmul(op_t, onesb_sb, st["absb"][c2],
                                 start=(c2 == 0), stop=(c2 == 1))

        def osb_piece(n):
            st = bstate.pop(n)
            osb = sb.tile([1, 512], F32, tag="osb", bufs=2, name="osb")
            nc.vector.tensor_tensor(osb, op_t, st["rr"], ALU.mult)
            nc.sync.dma_start(out=out_d[:, bass.ts(n, 512)], in_=osb)

        # ---- static piece schedule ------------------------------------
        from collections import defaultdict
        pieces = defaultdict(list)

        def sched(bi, w, fn, *a, **k):
            pieces[(bi, w)].append((fn, a, k))

        for i, jb in enumerate(range(1, 8)):
            sched(0, i, k_piece, jb)                 # b0 w0..w6
        sched(0, 7, q_piece, 1)
        for p in range(16):
            sched(0, 8 + p // 2, v_piece, p)         # b0 w8..w15, 2/window
        for ib in range(2, 8):
            sched(ib - 2, 12, q_piece, ib)           # q(ib) two blocks early
        for n in range(NBLK - 1):
            sched(n + 1, 6, cp_piece, n, 0)
            sched(n + 1, 10, cp_piece, n, 1)
            sched(n + 1, 13, outp_piece, n)
            sched(n + 1, 14, osb_piece, n)

        # pre-loop minimal projections (ACT copies; before EXPs exist)
        q_piece(0, act=True)
        k_piece(0, act=True)

        # ---- main loop -------------------------------------------------
        for n in range(NBLK):
            isl = bass.ts(n, 512)
            st = bstate.setdefault(n, {"est": {}})
            rtmp_prev = None
            rt2_list = []
            folds = 0
            for w in range(NWIN):
                stp = ps_stp.tile([128, 2, 512], F32, tag="stp", bufs=2,
                                  name="stp")
                for t in range(2):
                    jc = 2 * w + t
                    nc.tensor.matmul(
                        stp[:, t, :],
                        k4_sb[32 * t:32 * (t + 1), bass.ts(jc, 128)],
                        q4_sb[32 * t:32 * (t + 1), isl],
                        start=True, stop=True, tile_position=(32 * t, 0))
                est = sb.tile([128, 2, 512], BF16, tag="est", bufs=34,
                              name="est")
                nc.scalar.activation(est.rearrange("p a n -> p (a n)"),
                                     stp.rearrange("p a n -> p (a n)"),
                                     AF.Exp)
                st["est"][w] = est
                # r: DVE pair-sum planes, pair windows, exact PSUM fold
                rtmp = sb.tile([128, 512], BF16, tag="rtmp", bufs=3,
                               name="rtmp")
                nc.vector.tensor_tensor(rtmp, est[:, 0, :], est[:, 1, :],
                                        ALU.add)
                if rtmp_prev is None:
                    rtmp_prev = rtmp
                else:
                    rt2 = sb.tile([128, 512], BF16, tag="rt2", bufs=3,
                                  name="rt2")
                    nc.vector.tensor_tensor(rt2, rtmp_prev, rtmp, ALU.add)
                    rtmp_prev = None
                    rt2_list.append(rt2)
                # folds deferred past w3 so block n-1's rp reads drain
                while rt2_list and w >= 3:
                    nc.tensor.matmul(rp_t, onesb_sb,
                                     rt2_list.pop(0), start=(folds == 0),
                                     stop=(folds == NWIN // 2 - 1))
                    folds += 1
                for fn, a, k in pieces.pop((n, w), ()):
                    fn(*a, **k)
            # block tail: r products
            rr = sb.tile([1, 512], F32, tag="rr", bufs=2, name="rr")
            nc.vector.reciprocal_approx_fast(rr, rp_t)
            rlb = sb.tile([1, 512], BF16, tag="rlb", bufs=2, name="rlb")
            nc.vector.tensor_copy(rlb, rp_t)
            rb = sb.tile([128, 512], BF16, tag="rb", bufs=2, name="rb")
            nc.gpsimd.partition_broadcast(rb, rlb)
            x1r = sb.tile([128, 2, 512], BF16, tag="x1r", bufs=2,
                          name="x1r")
            for kc in range(2):
                nc.vector.tensor_tensor(x1r[:, kc, :], xc_sb[kc][:, isl],
                                        rb, ALU.mult)
            st.update(rr=rr, rlb=rlb, x1r=x1r)

        # tail: combine for the last block
        cp_piece(NBLK - 1, 0)
        cp_piece(NBLK - 1, 1)
        outp_piece(NBLK - 1)
        osb_piece(NBLK - 1)


_NC_CACHE = {}


def _get_nc():
    if "nc" not in _NC_CACHE:
        nc = bacc.Bacc("TRN2", debug=False, enable_asserts=False,
                       target_bir_lowering=False, enable_partition_id=False)
        with tile.TileContext(nc) as tc:
            build_program(nc, tc)
        nc.compile()
        _NC_CACHE["nc"] = nc
    return _NC_CACHE["nc"]


def host_inputs(x1, x2, Wq, bq, Wk, bk, Wv, bv, Wc, bc):
    """Build the 8 per-core input maps (host-side sharding/layout only)."""
    f = np.float32
    bf = ml_dtypes.bfloat16
    x1 = np.asarray(x1, f); x2 = np.asarray(x2, f)
    Wq = np.asarray(Wq, f); bq = np.asarray(bq, f)
    Wk = np.asarray(Wk, f)
    Wv = np.asarray(Wv, f); bv = np.asarray(bv, f)
    Wc = np.asarray(Wc, f); bc = np.asarray(bc, f)

    Wq4 = np.tile(Wq, (4, 1))            # [128, 256]
    Wk4 = np.tile(Wk, (4, 1))
    wqt = np.ascontiguousarray(Wq4.T.reshape(2, 128, 128)).astype(bf)
    wkt = np.ascontiguousarray(Wk4.T.reshape(2, 128, 128)).astype(bf)
    bq4 = np.tile(bq, 4).reshape(128, 1).copy()
    Wcx, Wca = Wc[:, :C], Wc[:, C:]
    W2 = Wca @ Wv                        # fold Wca into v projection
    wvt2 = np.ascontiguousarray(W2.T.reshape(2, 128, C)).astype(bf)
    wctx = np.ascontiguousarray(Wcx.T.reshape(2, 128, C)).astype(bf)
    bce = (bc + Wca @ bv).reshape(1, 2, 128).astype(bf)

    xs = [np.ascontiguousarray(x.reshape(B, 2, 128, N)).astype(bf)
          for x in (x1, x2)]
    in_maps = []
    for core in range(NCORES):
        b, br = divmod(core, 2)
        in_maps.append({
            "xa": xs[br][b], "xb": xs[1 - br][b], "xc": xs[0][b],
            "wqt": wqt, "wkt": wkt, "wvt2": wvt2, "wctx": wctx,
            "bq": bq4, "bce": bce,
        })
    return in_maps


def assemble(results):
    """results: 8 dicts with 'out' [1, N] -> (out1, out2) full."""
    outs = []
    for br in range(2):
        full = np.empty((B, 1, HH, WW), np.float32)
        for b in range(B):
            full[b, 0] = results[2 * b + br]["out"][0].reshape(HH, WW)
        outs.append(full)
    return outs[0], outs[1]


def kernel(x1, x2, Wq, bq, Wk, bk, Wv, bv, Wc, bc):
    in_maps = host_inputs(x1, x2, Wq, bq, Wk, bk, Wv, bv, Wc, bc)
    nc = _get_nc()
    res = run_bass_kernel_spmd(nc, in_maps, core_ids=list(range(NCORES)))
    return assemble(res.results)
